# revision 1
# baseline (speedup 1.0000x reference)
"""Trainium2 Bass kernel for nn_Classifier (GNN edge-MLP link predictor).

Computes, for E candidate edges:
    out[e] = W2 . relu( x_nc[i0[e]] @ W1[:H] + x_pr[i1[e]] @ W1[H:] + b1 ) + b2

Strategy (8 NeuronCores, data-parallel over edges):
  - Edges are sharded across the 8 cores (125k edges each).
  - Node tables are replicated, stored bf16 in DRAM.
  - Per tile of T edges: gpsimd.dma_gather(transpose=True) pulls the bf16
    feature rows for each endpoint directly into feature-major layout
    [H=128 partitions, T edges] in SBUF, so fc1 runs straight on the
    tensor engine with W1 as the stationary operand (no on-chip transpose).
  - relu(+b1) on ScalarE/VectorE (alternating), cast to bf16.
  - fc2 is a K=128, M=1 matmul; the [1, chunk] PSUM rows are drained
    (+b2) to SBUF by VectorE/ScalarE and DMAed out.

All matmuls are bf16 with fp32 PSUM accumulation (measured end-to-end
error vs the fp32 reference: ~4e-3 of output scale).
"""

import math

import numpy as np
import ml_dtypes

import concourse.bass as bass
import concourse.tile as tile
from concourse import bacc, mybir
from concourse import bass_utils

F32 = mybir.dt.float32
BF16 = mybir.dt.bfloat16
I16 = mybir.dt.int16

N_CORES = 8
H = 128

# Full-problem geometry (hardcoded per the task contract).
E_TOTAL = 1_000_000
N_NODES = 20_000


def _build(n_nodes: int, e_pad: int, t_gather: int, chunk: int, reps: int = 1):
    """Build + compile the per-core SPMD program.

    n_nodes: rows in each node table
    e_pad:   padded per-core edge count (multiple of t_gather)
    t_gather: edges per dma_gather instruction (multiple of 128)
    chunk:   edges per matmul (<=512, divides t_gather)
    reps:    repeat the edge loop (timing-harness use only)
    """
    assert e_pad % t_gather == 0 and t_gather % 128 == 0
    assert chunk <= 512 and t_gather % chunk == 0
    n_tiles = e_pad // t_gather

    nc = bacc.Bacc(
        "TRN2",
        target_bir_lowering=False,
        debug=False,
        num_devices=N_CORES,
    )

    t_nc = nc.dram_tensor("t_nc", [n_nodes, H], BF16, kind="ExternalInput").ap()
    t_pr = nc.dram_tensor("t_pr", [n_nodes, H], BF16, kind="ExternalInput").ap()
    idx0 = nc.dram_tensor("idx0", [16, e_pad // 16], I16, kind="ExternalInput").ap()
    idx1 = nc.dram_tensor("idx1", [16, e_pad // 16], I16, kind="ExternalInput").ap()
    w1 = nc.dram_tensor("w1", [2 * H, H], BF16, kind="ExternalInput").ap()
    b1 = nc.dram_tensor("b1", [H, 1], F32, kind="ExternalInput").ap()
    w2 = nc.dram_tensor("w2", [H, 1], BF16, kind="ExternalInput").ap()
    b2 = nc.dram_tensor("b2", [1, 1], F32, kind="ExternalInput").ap()
    out = nc.dram_tensor("out", [1, e_pad], F32, kind="ExternalOutput").ap()

    relu = mybir.ActivationFunctionType.Relu
    ident = mybir.ActivationFunctionType.Identity
    add_op = mybir.AluOpType.add
    max_op = mybir.AluOpType.max

    with tile.TileContext(nc) as tc:
        with (
            tc.tile_pool(name="const", bufs=1) as cpool,
            tc.tile_pool(name="idx", bufs=1) as ipool,
            tc.tile_pool(name="gather", bufs=2) as gpool,
            tc.tile_pool(name="h", bufs=2) as hpool,
            tc.tile_pool(name="stage", bufs=2) as spool,
            tc.tile_pool(name="fc1ps", bufs=4, space="PSUM") as fc1pool,
            tc.tile_pool(name="fc2ps", bufs=3, space="PSUM") as fc2pool,
        ):
            # ---- constants ----
            w1nc = cpool.tile([H, H], BF16, tag="w1nc")
            nc.sync.dma_start(w1nc[:], w1[0:H, :])
            w1pr = cpool.tile([H, H], BF16, tag="w1pr")
            nc.sync.dma_start(w1pr[:], w1[H : 2 * H, :])
            b1_sb = cpool.tile([H, 1], F32, tag="b1")
            nc.sync.dma_start(b1_sb[:], b1[:])
            w2_sb = cpool.tile([H, 1], BF16, tag="w2")
            nc.sync.dma_start(w2_sb[:], w2[:])
            b2_sb = cpool.tile([1, 1], F32, tag="b2")
            nc.sync.dma_start(b2_sb[:], b2[:])

            # ---- indices: replicate [16, N] across the 8 partition groups ----
            idx0_sb = ipool.tile([128, e_pad // 16], I16, tag="idx0")
            idx1_sb = ipool.tile([128, e_pad // 16], I16, tag="idx1")
            for k in range(8):
                nc.sync.dma_start(idx0_sb[16 * k : 16 * (k + 1), :], idx0[:])
                nc.sync.dma_start(idx1_sb[16 * k : 16 * (k + 1), :], idx1[:])

            ic = t_gather // 16  # idx columns per gather tile

            for t in [t for _ in range(reps) for t in range(n_tiles)]:
                g_nc = gpool.tile([H, t_gather], BF16, tag="g_nc")
                nc.gpsimd.dma_gather(
                    g_nc[:].rearrange("p (one t) -> p one t", one=1),
                    t_nc,
                    idx0_sb[:, t * ic : (t + 1) * ic],
                    t_gather,
                    t_gather,
                    H,
                    transpose=True,
                    single_packet=(t_gather <= 512),
                )
                g_pr = gpool.tile([H, t_gather], BF16, tag="g_pr")
                nc.gpsimd.dma_gather(
                    g_pr[:].rearrange("p (one t) -> p one t", one=1),
                    t_pr,
                    idx1_sb[:, t * ic : (t + 1) * ic],
                    t_gather,
                    t_gather,
                    H,
                    transpose=True,
                    single_packet=(t_gather <= 512),
                )

                h_sb = hpool.tile([H, t_gather], BF16, tag="h")
                stage = spool.tile([1, t_gather], F32, tag="stage")

                for c in range(t_gather // chunk):
                    sl = slice(c * chunk, (c + 1) * chunk)
                    ps = fc1pool.tile([H, chunk], F32, tag="fc1")
                    nc.tensor.matmul(
                        ps[:], w1nc[:], g_nc[:, sl], start=True, stop=False
                    )
                    nc.tensor.matmul(
                        ps[:], w1pr[:], g_pr[:, sl], start=False, stop=True
                    )
                    # relu(ps + b1) -> bf16, alternating engines
                    if c % 2 == 0:
                        nc.scalar.activation(h_sb[:, sl], ps[:], relu, bias=b1_sb[:])
                    else:
                        nc.vector.tensor_scalar(
                            h_sb[:, sl], ps[:], b1_sb[:], 0.0, add_op, max_op
                        )

                    ps2 = fc2pool.tile([1, chunk], F32, tag="fc2")
                    nc.tensor.matmul(
                        ps2[:], w2_sb[:], h_sb[:, sl], start=True, stop=True
                    )
                    # stage = ps2 + b2, opposite-parity engines
                    if c % 2 == 0:
                        nc.vector.tensor_scalar(
                            stage[:, sl], ps2[:], b2_sb[:], None, add_op
                        )
                    else:
                        nc.scalar.activation(stage[:, sl], ps2[:], ident, bias=b2_sb[:])

                nc.sync.dma_start(out[:, t * t_gather : (t + 1) * t_gather], stage[:])

    nc.compile()
    return nc


# ---------------------------------------------------------------------------
# Host-side wrapper
# ---------------------------------------------------------------------------

_CACHE: dict = {}


def _wrap_idx(idx: np.ndarray, e_pad: int) -> np.ndarray:
    """int16 [16, e_pad//16] with index i at [i % 16, i // 16]."""
    pad = np.zeros(e_pad, np.int16)
    pad[: idx.shape[0]] = idx.astype(np.int16)
    return np.ascontiguousarray(pad.reshape(e_pad // 16, 16).T)


def _get_program(n_nodes, e_pad, t_gather, chunk):
    key = (n_nodes, e_pad, t_gather, chunk)
    if key not in _CACHE:
        _CACHE[key] = _build(n_nodes, e_pad, t_gather, chunk)
    return _CACHE[key]


def kernel(
    x_ncRNA: np.ndarray,
    x_Protein: np.ndarray,
    edge_label_index: np.ndarray,
    W1: np.ndarray,
    b1: np.ndarray,
    W2: np.ndarray,
    b2: np.ndarray,
    _t_gather: int = 8192,
    _chunk: int = 512,
    _trace: bool = False,
) -> np.ndarray:
    E = edge_label_index.shape[1]
    n_nodes = x_ncRNA.shape[0]
    assert E % N_CORES == 0
    e_core = E // N_CORES
    n_tiles = math.ceil(e_core / _t_gather)
    e_pad = n_tiles * _t_gather

    nc = _get_program(n_nodes, e_pad, _t_gather, _chunk)

    t_nc = np.ascontiguousarray(x_ncRNA.astype(ml_dtypes.bfloat16))
    t_pr = np.ascontiguousarray(x_Protein.astype(ml_dtypes.bfloat16))
    w1 = np.ascontiguousarray(W1.astype(ml_dtypes.bfloat16))
    w2 = np.ascontiguousarray(W2.astype(ml_dtypes.bfloat16))
    b1_ = np.ascontiguousarray(b1.reshape(H, 1).astype(np.float32))
    b2_ = np.ascontiguousarray(b2.reshape(1, 1).astype(np.float32))

    in_maps = []
    for c in range(N_CORES):
        sl = slice(c * e_core, (c + 1) * e_core)
        in_maps.append(
            {
                "t_nc": t_nc,
                "t_pr": t_pr,
                "idx0": _wrap_idx(np.asarray(edge_label_index[0, sl]), e_pad),
                "idx1": _wrap_idx(np.asarray(edge_label_index[1, sl]), e_pad),
                "w1": w1,
                "b1": b1_,
                "w2": w2,
                "b2": b2_,
            }
        )

    res = bass_utils.run_bass_kernel_spmd(
        nc, in_maps, core_ids=list(range(N_CORES)), trace=_trace
    )
    out = np.empty(E, np.float32)
    for c in range(N_CORES):
        out[c * e_core : (c + 1) * e_core] = res.results[c]["out"][0, :e_core]
    kernel._last_results = res
    return out



# revision 5
# speedup vs baseline: 1.6774x; 1.6774x over previous
"""Trainium2 Bass kernel for nn_Classifier (GNN edge-MLP link predictor).

Computes, for E candidate edges:
    out[e] = W2 . relu( x_nc[i0[e]] @ W1[:H] + x_pr[i1[e]] @ W1[H:] + b1 ) + b2

Strategy (8 NeuronCores, data-parallel over edges).  The expensive part of
this problem is the per-edge gather: dma_gather costs ~7.8ns of gpsimd
descriptor generation per gathered row, so the kernel eliminates half of
all gathers via a one-hot matmul trick and keeps everything SBUF-resident:

  - Precompute on device: A = x_nc @ W1[:H] + b1  and  B = x_pr @ W1[H:]
    (20000x128 each, bf16), stored node-major in SBUF as [128, 157*128]
    (node n lives at partition n%128, free block n//128).
  - Edges are bucketed (host-side) by pr-endpoint window w = i1 >> 7.
    Each core's 125k edges are laid out in 157 window segments of CAP
    slots (tail-padded with idx -1).
  - B-side "gather" per 512-edge tile: build a one-hot [128, 512] matrix
    on-chip (K=1 broadcast matmul of the window-local index row, then an
    is_equal against an iota column) and multiply by the window's 128-row
    slice of B.  Zero descriptors; exact row selection.
  - A-side gather: SBUF-source dma_gather per window (trailing -1 padding
    is skipped by the ucode; the true count comes from a runtime register
    so one SPMD program serves all 8 cores).
  - h = relu(psum_B + gA) on DVE+ScalarE, fc2 = [128,1] matmul, +b2 drain.

Host reorders per-edge outputs back from slot order (index bookkeeping
only; all FLOPs and data movement stay on-device).
"""

import numpy as np
import ml_dtypes

import concourse.bass as bass
import concourse.tile as tile
from concourse import bacc, mybir
from concourse import bass_utils

F32 = mybir.dt.float32
BF16 = mybir.dt.bfloat16
I16 = mybir.dt.int16
I32 = mybir.dt.int32

N_CORES = 8
H = 128
N_NODES = 20_000
NW = (N_NODES + 127) // 128  # 157 windows of 128 pr-nodes
E_TOTAL = 1_000_000
TILE = 512


def _build(cap: int):
    """Build + compile the SPMD program (cap = slots per window segment)."""
    assert cap % TILE == 0
    n_slots = NW * cap
    nt = cap // TILE  # tiles per window

    nc = bacc.Bacc(
        "TRN2", target_bir_lowering=False, debug=False, num_devices=N_CORES
    )

    xT = nc.dram_tensor("xT", [H, 2 * N_NODES], BF16, kind="ExternalInput").ap()
    w1a = nc.dram_tensor("w1a", [H, H], BF16, kind="ExternalInput").ap()
    w1b = nc.dram_tensor("w1b", [H, H], BF16, kind="ExternalInput").ap()
    b1b = nc.dram_tensor("b1b", [H, H], F32, kind="ExternalInput").ap()
    w2 = nc.dram_tensor("w2", [H, 1], BF16, kind="ExternalInput").ap()
    b2 = nc.dram_tensor("b2", [1, 1], F32, kind="ExternalInput").ap()
    iota = nc.dram_tensor("iota", [H, 1], F32, kind="ExternalInput").ap()
    ones = nc.dram_tensor("ones", [1, H], BF16, kind="ExternalInput").ap()
    idx0 = nc.dram_tensor("idx0", [16, n_slots // 16], I16, kind="ExternalInput").ap()
    lidx = nc.dram_tensor("lidx", [NW, cap], BF16, kind="ExternalInput").ap()
    cnts = nc.dram_tensor("cnts", [1, NW], I32, kind="ExternalInput").ap()
    out = nc.dram_tensor("out", [1, n_slots], F32, kind="ExternalOutput").ap()

    relu = mybir.ActivationFunctionType.Relu
    ident = mybir.ActivationFunctionType.Identity
    add_op = mybir.AluOpType.add
    ieq = mybir.AluOpType.is_equal

    with tile.TileContext(nc) as tc:
        with (
            tc.tile_pool(name="const", bufs=1) as cpool,
            tc.tile_pool(name="x", bufs=2) as xpool,
            tc.tile_pool(name="g", bufs=3) as gpool,
            tc.tile_pool(name="oh", bufs=3) as ohpool,
            tc.tile_pool(name="h", bufs=3) as hpool,
            tc.tile_pool(name="l", bufs=2) as lpool,
            tc.tile_pool(name="st", bufs=2) as stpool,
            tc.tile_pool(name="pp", bufs=2, space="PSUM") as pppool,
            tc.tile_pool(name="bc", bufs=2, space="PSUM") as bcpool,
            tc.tile_pool(name="pb", bufs=2, space="PSUM") as pbpool,
            tc.tile_pool(name="p2", bufs=2, space="PSUM") as p2pool,
        ):
            # ---- constants ----
            w1a_sb = cpool.tile([H, H], BF16, tag="w1a")
            nc.sync.dma_start(w1a_sb[:], w1a[:])
            w1b_sb = cpool.tile([H, H], BF16, tag="w1b")
            nc.sync.dma_start(w1b_sb[:], w1b[:])
            b1b_sb = cpool.tile([H, H], F32, tag="b1b")
            nc.sync.dma_start(b1b_sb[:], b1b[:])
            w2_sb = cpool.tile([H, 1], BF16, tag="w2")
            nc.sync.dma_start(w2_sb[:], w2[:])
            b2_sb = cpool.tile([1, 1], F32, tag="b2")
            nc.sync.dma_start(b2_sb[:], b2[:])
            iota_sb = cpool.tile([H, 1], F32, tag="iota")
            nc.sync.dma_start(iota_sb[:], iota[:])
            ones_sb = cpool.tile([1, H], BF16, tag="ones")
            nc.sync.dma_start(ones_sb[:], ones[:])
            cnts_sb = cpool.tile([1, NW], I32, tag="cnts")
            nc.sync.dma_start(cnts_sb[:], cnts[:])
            idx0_sb = cpool.tile([128, n_slots // 16], I16, tag="idx0")
            for k in range(8):
                nc.sync.dma_start(idx0_sb[16 * k : 16 * (k + 1), :], idx0[:])

            # ---- node tables (node-major: node n -> partition n%128, block n//128)
            A_sb = cpool.tile([128, NW * H], BF16, tag="A")
            B_sb = cpool.tile([128, NW * H], BF16, tag="B")

            CH = 2048  # nodes per staged x chunk (16 blocks of 128)
            for tbl in range(2):  # 0: A (x_nc @ W1a + b1), 1: B (x_pr @ W1b)
                base = tbl * N_NODES
                wsb = w1a_sb if tbl == 0 else w1b_sb
                dst = A_sb if tbl == 0 else B_sb
                for off in range(0, N_NODES, CH):
                    cw = min(CH, N_NODES - off)
                    xc = xpool.tile([H, CH], BF16, tag="xt")
                    nc.sync.dma_start(xc[:, 0:cw], xT[:, base + off : base + off + cw])
                    for b in range(0, cw, 128):
                        c = (off + b) // 128
                        nn = min(128, cw - b)
                        ps = pppool.tile([128, H], F32, tag="pp")
                        nc.tensor.matmul(
                            ps[0:nn, :], xc[:, b : b + nn], wsb[:], start=True, stop=True
                        )
                        if tbl == 0:
                            nc.vector.tensor_tensor(
                                dst[0:nn, H * c : H * (c + 1)],
                                ps[0:nn, :],
                                b1b_sb[0:nn, :],
                                add_op,
                            )
                        else:
                            nc.scalar.activation(
                                dst[0:nn, H * c : H * (c + 1)], ps[0:nn, :], ident
                            )

            # ---- edge loop: one window (cap slots) at a time ----
            cnt_reg = nc.gpsimd.alloc_register("cnt_reg")
            for w in range(NW):
                kw = min(128, N_NODES - 128 * w)  # pr-nodes in this window
                nc.gpsimd.reg_load(cnt_reg, cnts_sb[0:1, w : w + 1])
                cnt_val = cnt_reg
                gA = gpool.tile([128, cap], BF16, tag="gA")
                nc.gpsimd.dma_gather(
                    gA[:].rearrange("p (one t) -> p one t", one=1),
                    A_sb[:],
                    idx0_sb[:, w * (cap // 16) : (w + 1) * (cap // 16)],
                    cap,
                    cnt_val,
                    H,
                    transpose=True,
                    single_packet=False,
                    sbuf_tokens_per_rank=128,
                    sbuf_free_dim_per_rank=2 * H,
                    sbuf_free_dim_pad_per_rank=0,
                    sbuf_byte_offset=0,
                )
                lrow = lpool.tile([1, cap], BF16, tag="lidx")
                nc.sync.dma_start(lrow[:], lidx[w : w + 1, :])
                st = stpool.tile([1, cap], F32, tag="st")
                for t in range(nt):
                    sl = slice(t * TILE, (t + 1) * TILE)
                    bc = bcpool.tile([128, TILE], F32, tag="bc")
                    nc.tensor.matmul(
                        bc[:], ones_sb[:], lrow[:, sl], start=True, stop=True
                    )
                    oh = ohpool.tile([128, TILE], BF16, tag="oh")
                    nc.vector.tensor_scalar(oh[:], bc[:], iota_sb[:], None, ieq)
                    pb = pbpool.tile([128, TILE], F32, tag="pb")
                    nc.tensor.matmul(
                        pb[:],
                        B_sb[0:kw, H * w : H * (w + 1)],
                        oh[0:kw, :],
                        start=True,
                        stop=True,
                    )
                    hpre = hpool.tile([128, TILE], BF16, tag="hpre")
                    nc.vector.scalar_tensor_tensor(
                        hpre[:], pb[:], 0.0, gA[:, sl], add_op, add_op
                    )
                    h = hpool.tile([128, TILE], BF16, tag="h")
                    nc.scalar.activation(h[:], hpre[:], relu)
                    p2 = p2pool.tile([1, TILE], F32, tag="p2")
                    nc.tensor.matmul(p2[:], w2_sb[:], h[:], start=True, stop=True)
                    nc.vector.tensor_scalar(st[:, sl], p2[:], b2_sb[:], None, add_op)
                nc.sync.dma_start(out[:, w * cap : (w + 1) * cap], st[:])

    nc.compile()
    return nc


# ---------------------------------------------------------------------------
# Host-side wrapper
# ---------------------------------------------------------------------------

_CACHE: dict = {}


def _get_program(cap: int):
    if cap not in _CACHE:
        _CACHE[cap] = _build(cap)
    return _CACHE[cap]


def kernel(
    x_ncRNA: np.ndarray,
    x_Protein: np.ndarray,
    edge_label_index: np.ndarray,
    W1: np.ndarray,
    b1: np.ndarray,
    W2: np.ndarray,
    b2: np.ndarray,
    _trace: bool = False,
) -> np.ndarray:
    E = edge_label_index.shape[1]
    n_nodes = x_ncRNA.shape[0]
    assert n_nodes == N_NODES and x_Protein.shape[0] == N_NODES
    assert E % N_CORES == 0
    e_core = E // N_CORES

    # ---- shared (replicated) inputs ----
    xT = np.ascontiguousarray(
        np.concatenate([x_ncRNA.T, x_Protein.T], axis=1).astype(ml_dtypes.bfloat16)
    )
    w1a = np.ascontiguousarray(W1[:H].astype(ml_dtypes.bfloat16))
    w1b = np.ascontiguousarray(W1[H:].astype(ml_dtypes.bfloat16))
    b1b = np.ascontiguousarray(np.tile(b1.reshape(1, H), (H, 1)).astype(np.float32))
    w2 = np.ascontiguousarray(W2.reshape(H, 1).astype(ml_dtypes.bfloat16))
    b2_ = np.ascontiguousarray(b2.reshape(1, 1).astype(np.float32))
    iota = np.arange(H, dtype=np.float32).reshape(H, 1)
    ones = np.ones((1, H), dtype=ml_dtypes.bfloat16)

    # ---- per-core bucketing by pr-window ----
    ei = np.asarray(edge_label_index)
    percore = []
    cap = 1024
    for c in range(N_CORES):
        sl = slice(c * e_core, (c + 1) * e_core)
        i0 = ei[0, sl].astype(np.int64)
        i1 = ei[1, sl].astype(np.int64)
        w = (i1 >> 7).astype(np.int64)
        order = np.argsort(w, kind="stable")
        cnts = np.bincount(w, minlength=NW).astype(np.int32)
        cap = max(cap, TILE * int(np.ceil(cnts.max() / TILE)))
        percore.append((i0, i1, w, order, cnts))

    n_slots = NW * cap
    nc = _get_program(cap)

    in_maps = []
    unpack = []
    for c in range(N_CORES):
        i0, i1, w, order, cnts = percore[c]
        starts = np.zeros(NW, np.int64)
        starts[1:] = np.cumsum(cnts)[:-1]
        # slot of the k-th sorted edge: w*cap + (k - start_w)
        ws = w[order]
        slots = ws * cap + (np.arange(e_core) - starts[ws])
        idx0_slots = np.full(n_slots, -1, np.int16)
        idx0_slots[slots] = i0[order].astype(np.int16)
        lidx_slots = np.zeros(n_slots, np.float32)
        lidx_slots[slots] = (i1[order] - (ws << 7)).astype(np.float32)
        in_maps.append(
            {
                "xT": xT,
                "w1a": w1a,
                "w1b": w1b,
                "b1b": b1b,
                "w2": w2,
                "b2": b2_,
                "iota": iota,
                "ones": ones,
                "idx0": np.ascontiguousarray(
                    idx0_slots.reshape(n_slots // 16, 16).T
                ),
                "lidx": np.ascontiguousarray(
                    lidx_slots.reshape(NW, cap).astype(ml_dtypes.bfloat16)
                ),
                "cnts": np.ascontiguousarray(cnts.reshape(1, NW)),
            }
        )
        unpack.append((order, slots))

    res = bass_utils.run_bass_kernel_spmd(
        nc, in_maps, core_ids=list(range(N_CORES)), trace=_trace
    )
    out = np.empty(E, np.float32)
    for c in range(N_CORES):
        order, slots = unpack[c]
        o = res.results[c]["out"].reshape(-1)
        seg = out[c * e_core : (c + 1) * e_core]
        seg[order] = o[slots]
    kernel._last_results = res
    return out
